# revision 7
# baseline (speedup 1.0000x reference)
"""
GroupedSelfAttention (GQA) Trainium2 Bass kernel, 8-way sharded.

Problem (hardcoded):
  x  [2, 2048, 1024] f32
  Wq [1024, 1024], bq [1024]
  Wk [1024, 128],  bk [128]     (2 KV groups x 64)
  Wv [1024, 128],  bv [128]
  Wo [1024, 1024], bo [1024]
  16 query heads x head_dim 64, 2 KV groups (8 heads/group), softmax scale 1/8.

Sharding: 8 cores = 2 batches x 4 query-head blocks (4 heads = 256 q-dims each;
each block lies inside one KV group, so its KV slice is just 64 dims).
Each core computes a partial output  x[b] -> (attn_out_block @ Wo[block_rows])
of shape [2048, 1024]; the host sums the 4 partials per batch and adds bo.

Per-core on-chip pipeline (all matmuls in float32r):
  - host passes x^T, so SBUF holds x^T [1024(dim), 2048(tok)] in 8 chunks of 128
  - Q^T [256, 2048], K^T [64->dup 128, 2048], V^T [64, 2048] via PSUM-accumulated
    matmuls over the 8 dim-chunks (bias added during PSUM->SBUF evac on DVE)
  - V natural [tok,64] via 16 PE transposes; augmented with a ones column ->
    Vaug [128, 65] so the attention-output matmul also produces the softmax
    denominators for free (row 64 of its PSUM tile)
  - attention, streamed per (head-pair j, 512-wide query tile qt):
      for each of 16 key chunks: scores^T [k=128, q=512] for both heads of the
      pair in one row-tiled concurrent matmul pair -> ACT exp (scale=1/8)
      -> two accumulating matmuls (Vaug^T @ expS) into [65, 512] PSUM tiles
    epilogue: DVE reciprocal of the denominator rows, PE broadcast of the
    reciprocals across 64 partitions, DVE normalize, h1 half moved to
    partitions 64..127 by an SBUF->SBUF DMA -> attnT [128, 2048] per j
  - output projection: out[tok, e] accumulated over the two 128-dim chunks of
    attnT with Wo row-slices, evacuated and DMA'd to DRAM.
"""

import os
import numpy as np

import concourse.bass as bass
import concourse.bacc as bacc
import concourse.mybir as mybir
from contextlib import ExitStack
from concourse.tile import TileContext
from concourse.bass_utils import run_bass_kernel_spmd

F32 = mybir.dt.float32
F32R = mybir.dt.float32r
EXP = mybir.ActivationFunctionType.Exp

DIM = 1024
S = 2048
QBLK = 256          # q-dims per core (4 heads)
KVB = 64            # kv-dims per core (1 group slice)
NCHUNK = DIM // 128  # 8 contraction chunks for projections
NT = S // 128        # 16 token chunks of 128
NQ = S // 512        # 4 query tiles of 512
MM_DT = os.environ.get("KERNEL_MM_DT", "f32r")  # f32r | f32 | bf16


DT = F32R if MM_DT == "f32r" else F32


def _mm(ap):
    return ap


def _build_nc():
    nc = bacc.Bacc("TRN2", target_bir_lowering=False)

    xt = nc.dram_tensor("xt", [DIM, S], DT, kind="ExternalInput")
    wq = nc.dram_tensor("wq", [DIM, QBLK], DT, kind="ExternalInput")
    wk = nc.dram_tensor("wk", [DIM, KVB], DT, kind="ExternalInput")
    wv = nc.dram_tensor("wv", [DIM, KVB], DT, kind="ExternalInput")
    wo = nc.dram_tensor("wo", [QBLK, DIM], DT, kind="ExternalInput")
    bq = nc.dram_tensor("bq2", [128, 2], F32, kind="ExternalInput")
    bk = nc.dram_tensor("bk1", [KVB, 1], F32, kind="ExternalInput")
    bv = nc.dram_tensor("bv1", [KVB, 1], F32, kind="ExternalInput")
    ident = nc.dram_tensor("ident", [128, 128], F32, kind="ExternalInput")
    ones_row = nc.dram_tensor("ones_row", [1, S], DT, kind="ExternalInput")
    onesf = nc.dram_tensor("onesf", [1, S], F32, kind="ExternalInput")
    out = nc.dram_tensor("out", [S, DIM], F32, kind="ExternalOutput")

    with TileContext(nc) as tc, ExitStack() as ctx:
        sg = ctx.enter_context(tc.tile_pool(name="sg", bufs=1))
        psP = ctx.enter_context(tc.tile_pool(name="psP", bufs=2, space="PSUM"))
        psS = ctx.enter_context(tc.tile_pool(name="psS", bufs=2, space="PSUM"))
        psO = ctx.enter_context(tc.tile_pool(name="psO", bufs=1, space="PSUM"))
        exP = ctx.enter_context(tc.tile_pool(name="exP", bufs=3))
        evP = ctx.enter_context(tc.tile_pool(name="evP", bufs=2))
        outP = ctx.enter_context(tc.tile_pool(name="outP", bufs=3))

        # ---- persistent SBUF tiles ----
        xt_sb = sg.tile([128, NCHUNK * S], DT, name="xt_sb")
        wq_sb = sg.tile([128, NCHUNK * QBLK], DT, name="wq_sb")
        wk_sb = sg.tile([128, NCHUNK * KVB], DT, name="wk_sb")
        wv_sb = sg.tile([128, NCHUNK * KVB], DT, name="wv_sb")
        wo_sb = sg.tile([128, 2 * DIM], DT, name="wo_sb")
        qt_sb = sg.tile([128, 2 * S], DT, name="qt_sb")
        kt_sb = sg.tile([128, S], DT, name="kt_sb")
        vt_sb = sg.tile([KVB + 1, S], F32, name="vt_sb")
        attnT = sg.tile([128, 2 * S], DT, name="attnT")
        id_sb = sg.tile([128, 128], F32, name="id_sb")
        on_sb = sg.tile([65, 64], DT, name="on_sb")
        bq_sb = sg.tile([128, 2], F32, name="bq_sb")
        bk_sb = sg.tile([KVB, 1], F32, name="bk_sb")
        bv_sb = sg.tile([KVB, 1], F32, name="bv_sb")

        # ---- input DMAs ----
        nc.sync.dma_start(out=id_sb[:], in_=ident[:])
        nc.sync.dma_start(out=bq_sb[:], in_=bq[:])
        nc.sync.dma_start(out=bk_sb[:], in_=bk[:])
        nc.sync.dma_start(out=bv_sb[:], in_=bv[:])
        def chunked(dram, width, n):
            return bass.AP(dram[:].tensor, 0,
                           [[width, 128], [128 * width, n], [1, width]])

        nc.sync.dma_start(out=wk_sb[:].rearrange("p (c f) -> p c f", c=NCHUNK),
                          in_=chunked(wk, KVB, NCHUNK))
        nc.sync.dma_start(out=wv_sb[:].rearrange("p (c f) -> p c f", c=NCHUNK),
                          in_=chunked(wv, KVB, NCHUNK))
        nc.sync.dma_start(out=wq_sb[:].rearrange("p (c f) -> p c f", c=NCHUNK),
                          in_=chunked(wq, QBLK, NCHUNK))
        nc.sync.dma_start(out=wo_sb[:].rearrange("p (c f) -> p c f", c=2),
                          in_=chunked(wo, DIM, 2))
        nc.sync.dma_start(out=xt_sb[:].rearrange("p (c f) -> p c f", c=NCHUNK),
                          in_=chunked(xt, S, NCHUNK))

        nc.sync.dma_start(out=vt_sb[KVB:KVB + 1, :], in_=onesf[:])
        nc.sync.dma_start(out=on_sb[64:65, :], in_=ones_row[0:1, 0:64])

        def xslice(c, s):
            return xt_sb[:, c * S + s * 512: c * S + s * 512 + 512]

        # ---- K^T projection (+ duplicate to partitions 64..127) ----
        for s in range(4):
            ps = psP.tile([KVB, 512], F32, tag="ps", name="ps")
            for c in range(NCHUNK):
                nc.tensor.matmul(ps[:], _mm(wk_sb[:, c * KVB:(c + 1) * KVB]),
                                 _mm(xslice(c, s)),
                                 start=(c == 0), stop=(c == NCHUNK - 1))
            t = slice(s * 512, (s + 1) * 512)
            nc.vector.tensor_scalar_add(kt_sb[0:64, t], ps[:], bk_sb[:])
            nc.sync.dma_start(out=kt_sb[64:128, t], in_=kt_sb[0:64, t])

        # ---- V^T projection ----
        for s in range(4):
            ps = psP.tile([KVB, 512], F32, tag="ps", name="ps")
            for c in range(NCHUNK):
                nc.tensor.matmul(ps[:], _mm(wv_sb[:, c * KVB:(c + 1) * KVB]),
                                 _mm(xslice(c, s)),
                                 start=(c == 0), stop=(c == NCHUNK - 1))
            nc.vector.tensor_scalar_add(vt_sb[0:KVB, s * 512:(s + 1) * 512], ps[:], bv_sb[:])

        # ---- Q^T projection ----
        for j in range(2):
            for s in range(4):
                ps = psP.tile([128, 512], F32, tag="ps", name="ps")
                for c in range(NCHUNK):
                    w = wq_sb[:, c * QBLK + j * 128: c * QBLK + j * 128 + 128]
                    nc.tensor.matmul(ps[:], _mm(w), _mm(xslice(c, s)),
                                     start=(c == 0), stop=(c == NCHUNK - 1))
                nc.vector.tensor_scalar_add(
                    qt_sb[:, j * S + s * 512: j * S + s * 512 + 512],
                    ps[:], bq_sb[:, j:j + 1])

        # ---- V natural [tok, 64] + ones column -> Vaug [128, 65] ----
        va_tiles = []
        for t in range(NT):
            pst = psP.tile([128, KVB + 1], F32, tag="ps", name="pst")
            nc.tensor.transpose(pst[:], vt_sb[:, t * 128:(t + 1) * 128],
                                id_sb[0:KVB + 1, 0:KVB + 1])
            va = sg.tile([128, 68], DT, tag=f"vaug{t}", name=f"va{t}")
            nc.vector.tensor_copy(va[:, 0:KVB + 1], pst[:])
            va_tiles.append(va)

        # ---- attention ----
        for j in range(2):
            for qt in range(NQ):
                q0 = qt_sb[0:64, j * S + qt * 512: j * S + qt * 512 + 512]
                q1 = qt_sb[64:128, j * S + qt * 512: j * S + qt * 512 + 512]
                o0 = psO.tile([65, 512], F32, tag="o0", name="o0")
                o1 = psO.tile([65, 512], F32, tag="o1", name="o1")
                for c in range(NT):
                    k = slice(c * 128, (c + 1) * 128)
                    sc = psS.tile([128, 1024], F32, tag="sc", name="sc")
                    nc.tensor.matmul(sc[:, 0:512], _mm(kt_sb[0:64, k]), _mm(q0),
                                     tile_position=(0, 0))
                    nc.tensor.matmul(sc[:, 512:1024], _mm(kt_sb[64:128, k]), _mm(q1),
                                     tile_position=(64, 0))
                    ex = exP.tile([128, 1024], DT, tag="ex", name="ex")
                    nc.scalar.activation(ex[:], sc[:], EXP, bias=0.0, scale=0.125)
                    nc.tensor.matmul(o0[:], _mm(va_tiles[c][:, 0:65]), _mm(ex[:, 0:512]),
                                     start=(c == 0), stop=(c == NT - 1),
                                     skip_group_check=True)
                    nc.tensor.matmul(o1[:], _mm(va_tiles[c][:, 0:65]), _mm(ex[:, 512:1024]),
                                     start=(c == 0), stop=(c == NT - 1),
                                     skip_group_check=True)
                # epilogue: normalize by the denominators sitting in row 64
                rp = evP.tile([65, 1024], DT, tag="rp", name="rp")
                with nc.allow_low_precision(reason="f32r softmax denominators"):
                    nc.vector.reciprocal(rp[64:65, 0:512], o0[64:65, :])
                    nc.vector.reciprocal(rp[64:65, 512:1024], o1[64:65, :])
                pb = psS.tile([128, 1024], F32, tag="sc", name="pb")
                nc.tensor.matmul(pb[0:64, 0:512], _mm(on_sb[64:65, :]),
                                 _mm(rp[64:65, 0:512]), tile_position=(64, 0))
                nc.tensor.matmul(pb[0:64, 512:1024], _mm(on_sb[64:65, :]),
                                 _mm(rp[64:65, 512:1024]), tile_position=(64, 0))
                bc = evP.tile([64, 1024], F32, tag="bc", name="bc")
                nc.vector.tensor_copy(bc[:], pb[0:64, :])
                t = slice(j * S + qt * 512, j * S + qt * 512 + 512)
                nc.vector.tensor_mul(attnT[0:64, t], o0[0:64, :], bc[:, 0:512])
                tm = evP.tile([64, 512], DT, tag="tm", name="tm")
                nc.vector.tensor_mul(tm[:], o1[0:64, :], bc[:, 512:1024])
                nc.sync.dma_start(out=attnT[64:128, t], in_=tm[:])

        # ---- output projection (partial: this core's 256 dims only) ----
        for t in range(NT):
            for e in range(2):
                ps = psP.tile([128, 512], F32, tag="ps", name="ps")
                for j in range(2):
                    lhs = attnT[:, j * S + t * 128: j * S + (t + 1) * 128]
                    rhs = wo_sb[:, j * DIM + e * 512: j * DIM + e * 512 + 512]
                    nc.tensor.matmul(ps[:], _mm(lhs), _mm(rhs),
                                     start=(j == 0), stop=(j == 1))
                ob = outP.tile([128, 512], F32, tag="ob", name="ob")
                nc.vector.tensor_copy(ob[:], ps[:])
                nc.sync.dma_start(out=out[t * 128:(t + 1) * 128,
                                          e * 512:(e + 1) * 512], in_=ob[:])

    nc.finalize()
    return nc


_NC = None
LAST_RESULT = None


def _get_nc():
    global _NC
    if _NC is None:
        _NC = _build_nc()
    return _NC


def kernel(x, Wq, bq, Wk, bk, Wv, bv, Wo, bo):
    global LAST_RESULT
    x = np.asarray(x, dtype=np.float32)
    Wq = np.asarray(Wq, dtype=np.float32)
    bq = np.asarray(bq, dtype=np.float32)
    Wk = np.asarray(Wk, dtype=np.float32)
    bk = np.asarray(bk, dtype=np.float32)
    Wv = np.asarray(Wv, dtype=np.float32)
    bv = np.asarray(bv, dtype=np.float32)
    Wo = np.asarray(Wo, dtype=np.float32)
    bo = np.asarray(bo, dtype=np.float32)

    nc = _get_nc()
    ident = np.eye(128, dtype=np.float32)
    in_maps = []
    for core in range(8):
        b, blk = divmod(core, 4)
        g = blk // 2
        qs = slice(blk * QBLK, (blk + 1) * QBLK)
        ks = slice(g * KVB, (g + 1) * KVB)
        in_maps.append({
            "xt": np.ascontiguousarray(x[b].T),
            "wq": np.ascontiguousarray(Wq[:, qs]),
            "wk": np.ascontiguousarray(Wk[:, ks]),
            "wv": np.ascontiguousarray(Wv[:, ks]),
            "wo": np.ascontiguousarray(Wo[qs, :]),
            "bq2": np.ascontiguousarray(bq[qs].reshape(2, 128).T),
            "bk1": np.ascontiguousarray(bk[ks].reshape(KVB, 1)),
            "bv1": np.ascontiguousarray(bv[ks].reshape(KVB, 1)),
            "ident": ident,
            "ones_row": np.ones((1, S), dtype=np.float32),
            "onesf": np.ones((1, S), dtype=np.float32),
        })

    LAST_RESULT = run_bass_kernel_spmd(nc, in_maps, core_ids=list(range(8)))
    outs = [r["out"] for r in LAST_RESULT.results]

    y = np.empty((2, S, DIM), dtype=np.float32)
    for b in range(2):
        y[b] = outs[4 * b] + outs[4 * b + 1] + outs[4 * b + 2] + outs[4 * b + 3] + bo
    return y
